# revision 3
# baseline (speedup 1.0000x reference)
"""Trainium2 Bass kernel v2 for nn_LoopVisibleLSTM (T=2048, B=32, D=256, H=256, L=2).

Transposed-gates design: gates live in PSUM as gates.T chunks
[128 gate-units, batch] so the per-step sigmoid is a [128, 256]-class op
(all 128 partitions busy) instead of [32, 1024], and the elementwise cell
update runs on [128, 64] tiles.  The recurrent matmul keeps Whh STATIONARY
(fp8 e3m4, FWL fast weight load) and streams h.T (fp8) as the 32-column
moving operand; the weights transit the PE once per step per layer, which
is the irreducible cost.  Input-side gate projections are bulk-computed per
8-step group directly from the (host-pre-transposed) input with W_init
folded into Wih0 on the host; biases ride in via a tiny K=2 matmul.
tanh is eliminated everywhere via tanh(z) = 2*sigmoid(2z)-1 with the 2s
folded into weights/host post-scaling, so ACT only ever runs Sigmoid.
Gate sections are stored in order [g i f o] so sigmoid(g,i) is one strided
op and the o-gate arrives last, matching the dependency chain
  MM -> sig(g,i) -> i*g -> f*c -> c' -> sig(2c') -> h'.
Output h1 is staged transposed (bf16) and un-transposed on the host.
"""

import sys
import os

for _p in ("/opt/pypackages", "/opt/trn_rl_repo"):
    if _p not in sys.path:
        sys.path.insert(0, _p)

import numpy as np

T_FULL, B, D, H = 2048, 32, 256, 256
G = 8            # steps per gates PSUM group
HALF = 32        # steps per half-block (input/output DMA staging)
BODY = 64        # steps per For_i body (8 groups, 2 half-blocks)
P_SC = 256.0     # PSUM gate pre-activation scale


def build(T, loops=1):
    import concourse.bass as bass
    import concourse.mybir as mybir
    from concourse import bacc

    FP32 = mybir.dt.float32
    BF16 = mybir.dt.bfloat16
    F16 = mybir.dt.float16
    FP8 = mybir.dt.float8e3
    AF = mybir.ActivationFunctionType
    ALU = mybir.AluOpType

    assert T % BODY == 0
    n_body = T // BODY

    nc = bacc.Bacc("TRN2", target_bir_lowering=False, debug=False)

    # ---------------- DRAM parameters ----------------
    # input.T, padded by one body's columns so the prefetch DMA of the
    # (nonexistent) body after the last one stays in bounds
    inpT = nc.declare_dram_parameter("inputT", [256, T * B + 2 * HALF * B], BF16,
                                     isOutput=False)
    whh8_d = [nc.declare_dram_parameter(f"whh8_{l}", [128, 2048], FP8,
                                        isOutput=False) for l in range(2)]
    wihb_d = [nc.declare_dram_parameter(f"wihb_{l}", [128, 2048], BF16,
                                        isOutput=False) for l in range(2)]
    biasm_d = [nc.declare_dram_parameter(f"biasm_{l}", [4, 256], BF16,
                                         isOutput=False) for l in range(2)]
    onehot_d = nc.declare_dram_parameter("onehot4", [4, 512], BF16, isOutput=False)
    h8i_d = [nc.declare_dram_parameter(f"h8i_{l}", [128, 64], FP8,
                                       isOutput=False) for l in range(2)]
    ci_d = [nc.declare_dram_parameter(f"ci_{l}", [128, 64], FP32,
                                      isOutput=False) for l in range(2)]
    fwdT = nc.declare_dram_parameter("fwdT", [256, T * B], BF16, isOutput=True)

    ctxs = []

    def sb(shape, dtype=FP32):
        cm = nc.sbuf_tensor(shape, dtype)
        t = cm.__enter__()
        ctxs.append(cm)
        return t

    def ps(shape, dtype=FP32):
        cm = nc.psum_tensor(shape, dtype)
        t = cm.__enter__()
        ctxs.append(cm)
        return t

    # ---------------- SBUF constants ----------------
    whh8 = [sb([128, 2048], FP8) for _ in range(2)]    # tiles (hc, q) at 128*(hc*8+q)
    wihb = [sb([128, 2048], BF16) for _ in range(2)]   # tiles (dk, q)
    biasm = [sb([4, 256], BF16) for _ in range(2)]     # [4, 128] per bank-half
    onehot = sb([4, 512], BF16)                        # K=4 one-hot

    # ---------------- SBUF state / staging ----------------
    inT = [[sb([128, 1024], BF16) for _ in range(2)] for _ in range(2)]  # [dk][slot]
    hT8 = [[sb([128, 64], FP8) for _ in range(2)] for _ in range(2)]     # [l][parity]
    cst = [[sb([128, 64], FP32) for _ in range(2)] for _ in range(2)]    # [l][parity]
    sbf = [[sb([128, 256], F16) for _ in range(2)] for _ in range(2)]    # sigmoid out
    igb = [[sb([128, 64], FP32) for _ in range(2)] for _ in range(2)]
    fcb = [[sb([128, 64], FP32) for _ in range(2)] for _ in range(2)]
    s2c = [[sb([128, 64], F16) for _ in range(2)] for _ in range(2)]
    h0g = [sb([128, 512], BF16) for _ in range(2)]     # [group parity]: (hc, s, b)
    stg = [sb([128, 2048], BF16) for _ in range(2)]    # [hb slot]: (hc, t_loc, b)

    # ---------------- PSUM: gates.T, (q, s, b) column order ----------------
    gp = [ps([128, 2048]) for _ in range(2)]           # 4 banks per layer

    import concourse.tile as tile_mod

    with tile_mod.TileContext(nc) as tc:
        dma = nc.sync

        # ------------ constant + init loads ------------
        for l in range(2):
            dma.dma_start(whh8[l][:, :], whh8_d[l][:, :])
            dma.dma_start(wihb[l][:, :], wihb_d[l][:, :])
            dma.dma_start(biasm[l][:, :], biasm_d[l][:, :])
            dma.dma_start(hT8[l][0][:, :], h8i_d[l][:, :])
            dma.dma_start(cst[l][0][:, :], ci_d[l][:, :])
        dma.dma_start(onehot[:, :], onehot_d[:, :])
        dma.dma_start(inT[0][0][:, :], inpT[0:128, 0:1024])
        dma.dma_start(inT[1][0][:, :], inpT[128:256, 0:1024])

        def emit_gin_slice(l, g, half, bh):
            """Bias + input-side projection for bank 2*half+bh of group g
            into gp[l].  Column layout: col = 1024*half + 128*q + 32*s' + b
            (s' = step within half), so each half owns its two banks and the
            single K=4 one-hot bias matmul is the bank's only start=True
            writer (PSUM start zeroes whole 2KB banks)."""
            bank = 2 * half + bh
            nc.tensor.matmul(
                gp[l][:, 512 * bank:512 * (bank + 1)],
                biasm[l][:, 128 * bh:128 * (bh + 1)],
                onehot[:, :],
                start=True, stop=False, skip_group_check=True,
            )
            for dk in range(2):
                for q in range(4 * bh, 4 * bh + 4):
                    if l == 0:
                        rhs = inT[dk][(g // 4) % 2][
                            :, 256 * (g % 4) + 128 * half:
                            256 * (g % 4) + 128 * (half + 1)]
                    else:
                        rhs = h0g[g % 2][:, 256 * dk + 128 * half:
                                         256 * dk + 128 * (half + 1)]
                    nc.tensor.matmul(
                        gp[l][:, 1024 * half + 128 * q:1024 * half + 128 * (q + 1)],
                        wihb[l][:, 128 * (dk * 8 + q):128 * (dk * 8 + q + 1)],
                        rhs,
                        start=False, stop=(dk == 1), skip_group_check=True,
                    )

        def emit_step_front(l, j):
            """Recurrent matmul + gate sigmoids + cell update for step j."""
            p = j % 2
            pn = (j + 1) % 2
            base = 1024 * (j // 4) + 32 * (j % 4)
            # 16 recurrent tiles: q order g(0,1) i(2,3) f(4,5) o(6,7)
            for q in range(8):
                for hc in range(2):
                    nc.tensor.matmul(
                        gp[l][:, base + 128 * q:base + 128 * q + 32],
                        whh8[l][:, 128 * (hc * 8 + q):128 * (hc * 8 + q + 1)],
                        hT8[l][p][:, 32 * hc:32 * (hc + 1)],
                        start=False, stop=(q == 7 and hc == 1),
                        skip_group_check=True,
                    )
            gv = gp[l][:, 1024 * (j // 4):1024 * (j // 4) + 1024].rearrange(
                "p (q r) -> p q r", q=8)
            sv = sbf[l][p][:, :].rearrange("p (q b) -> p q b", q=8)
            # one sigmoid over all four gate sections [g i f o]
            nc.scalar.activation(sv[:, :, :], gv[:, :, 32 * (j % 4):32 * (j % 4) + 32],
                                 AF.Sigmoid, scale=1.0 / P_SC)
            s_ = sbf[l][p]
            # ig = (sg - 0.5) * si ;  fc = sf * c ;  c' = 2*ig + fc
            nc.vector.scalar_tensor_tensor(
                igb[l][p][:, :], s_[:, 0:64], 0.5, s_[:, 64:128],
                ALU.subtract, ALU.mult)
            nc.vector.tensor_mul(fcb[l][p][:, :], s_[:, 128:192], cst[l][p][:, :])
            nc.vector.scalar_tensor_tensor(
                cst[l][pn][:, :], igb[l][p][:, :], 2.0, fcb[l][p][:, :],
                ALU.mult, ALU.add)

        def emit_step_back(l, j, g):
            """tanh(c') and the two h' products (fp8 recurrent + bf16 tap)."""
            p = j % 2
            pn = (j + 1) % 2
            s_ = sbf[l][p]
            nc.scalar.activation(s2c[l][p][:, :], cst[l][pn][:, :], AF.Tanh)
            # recurrent h (fp8, = 16*h'): (tanh_c * 16) * so
            nc.vector.scalar_tensor_tensor(
                hT8[l][pn][:, :], s2c[l][p][:, :], 16.0, s_[:, 192:256],
                ALU.mult, ALU.mult)
            # bf16 tap (= h'): layer0 -> next-layer group buffer,
            # layer1 -> output staging
            if l == 0:
                dst = h0g[g % 2][:, :].rearrange(
                    "p (h sb) -> p h sb", h=2)[:, :, 32 * j:32 * (j + 1)]
            else:
                t_loc = (g % 4) * G + j
                dst = stg[(g // 4) % 2][:, :].rearrange(
                    "p (h sb) -> p h sb", h=2)[:, :, 32 * t_loc:32 * (t_loc + 1)]
            src2 = s2c[l][p][:, :].rearrange("p (h b) -> p h b", h=2)
            srco = s_[:, 192:256].rearrange("p (h b) -> p h b", h=2)
            nc.gpsimd.tensor_mul(dst, src2, srco)

        def emit_out_dma(hb, i):
            sl = hb % 2
            col = i + HALF * B * hb
            dma.dma_start(fwdT[0:128, bass.ds(col, 1024)], stg[sl][:, 0:1024])
            dma.dma_start(fwdT[128:256, bass.ds(col, 1024)], stg[sl][:, 1024:2048])

        def emit_body(i, last=False):
            for gg in range(9):
                g0 = gg          # layer-0 group
                g1 = gg - 1      # layer-1 group (lags one group)
                if g0 == 0:
                    # prefetch second half-block of this body
                    dma.dma_start(inT[0][1][:, :],
                                  inpT[0:128, bass.ds(i + 1024, 1024)])
                    dma.dma_start(inT[1][1][:, :],
                                  inpT[128:256, bass.ds(i + 1024, 1024)])
                if g0 == 4:
                    # prefetch first half-block of the NEXT body (padded DRAM)
                    dma.dma_start(inT[0][0][:, :],
                                  inpT[0:128, bass.ds(i + 2048, 1024)])
                    dma.dma_start(inT[1][0][:, :],
                                  inpT[128:256, bass.ds(i + 2048, 1024)])
                for j in range(G):
                    if g0 < 8:
                        emit_step_front(0, j)
                    if g1 >= 0:
                        emit_step_front(1, j)
                    if g0 < 8:
                        emit_step_back(0, j, g0)
                    if g1 >= 0:
                        emit_step_back(1, j, g1)
                    # gin sub-slices, interleaved with the steps so the PE
                    # never sees a multi-us projection lump.  j<4: half 1 of
                    # the groups currently running (their steps 4-7); j>=4:
                    # half 0 of the NEXT groups (g0+1==8 is group 0 of the
                    # next body; inT slot 0 holds its prefetched input).
                    if j in (0, 1):
                        if g0 <= 8 and not (last and g0 == 8):
                            emit_gin_slice(0, g0 % 8, 1, j)
                        if g1 >= 0:
                            emit_gin_slice(1, g1, 1, j)
                    elif j in (4, 5):
                        if g0 < 8 and not (last and g0 == 7):
                            emit_gin_slice(0, (g0 + 1) % 8, 0, j - 4)
                        if 0 <= g0 <= 7:
                            emit_gin_slice(1, g0, 0, j - 4)
                if g1 >= 0 and g1 % 4 == 3:
                    emit_out_dma(g1 // 4, i)

        # group-0 half-0 gin of the first body runs before the loop
        for bh in range(2):
            emit_gin_slice(0, 0, 0, bh)
        if n_body == 1:
            emit_body(0, last=True)
        elif loops == 1:
            with tc.For_i(0, T * B, BODY * B) as i:
                emit_body(i)
        else:
            # timing-only variant: run the whole sequence `loops` times to
            # amortize fixed dispatch overhead (outputs are overwritten)
            with tc.For_i(0, loops, 1):
                with tc.For_i(0, T * B, BODY * B) as i:
                    emit_body(i)

    for cm in reversed(ctxs):
        cm.__exit__(None, None, None)

    nc.compile()
    return nc


def prep_inputs(inputs, T):
    """Host-side reparameterization (weights) + input transpose."""
    import ml_dtypes
    E3M4 = ml_dtypes.float8_e3m4
    BF16 = ml_dtypes.bfloat16

    Wih = inputs["Wih"].astype(np.float32)
    Whh = inputs["Whh"].astype(np.float32)
    bih = inputs["bih"].astype(np.float32)
    bhh = inputs["bhh"].astype(np.float32)
    W_init = inputs["W_init"].astype(np.float32)
    b_init = inputs["b_init"].astype(np.float32)
    h0 = inputs["h0"].astype(np.float32)
    c0 = inputs["c0"].astype(np.float32)
    inp = np.ascontiguousarray(inputs["input"], np.float32)

    # stored gate-section order [g i f o]; g rows doubled (tanh folding)
    perm = np.concatenate([np.arange(512, 768), np.arange(0, 256),
                           np.arange(256, 512), np.arange(768, 1024)])
    R = np.ones((1024, 1), np.float32)
    R[512:768] = 2.0

    im = {}
    for l in range(2):
        if l == 0:
            Wihp = Wih[0] @ W_init                 # fold init projection
            biasp = bih[0] + bhh[0] + Wih[0] @ b_init
        else:
            Wihp = Wih[1]                          # h0 tap is h (full scale)
            biasp = bih[1] + bhh[1]
        Wr = (P_SC * R * Wihp)[perm]               # [1024 stored, 256 d]
        Whr = ((P_SC / 16.0) * R * Whh[l])[perm]   # [1024 stored, 256 h]

        wih_sb = np.empty((128, 2048), np.float32)
        whh_sb = np.empty((128, 2048), np.float32)
        for k in range(2):
            for q in range(8):
                wih_sb[:, 128 * (k * 8 + q):128 * (k * 8 + q + 1)] = \
                    Wr[128 * q:128 * (q + 1), 128 * k:128 * (k + 1)].T
                whh_sb[:, 128 * (k * 8 + q):128 * (k * 8 + q + 1)] = \
                    Whr[128 * q:128 * (q + 1), 128 * k:128 * (k + 1)].T
        im[f"wihb_{l}"] = wih_sb.astype(BF16)
        im[f"whh8_{l}"] = whh_sb.astype(E3M4)

        bs = (P_SC * R[:, 0] * biasp)[perm]        # [1024]
        bm = np.empty((4, 256), np.float32)
        for bh in range(2):
            for r in range(4):
                bm[r, 128 * bh:128 * (bh + 1)] = \
                    bs[128 * (4 * bh + r):128 * (4 * bh + r + 1)]
        im[f"biasm_{l}"] = bm.astype(BF16)

        # initial states in (hc, b) layout
        h8 = np.empty((128, 64), np.float32)
        ci = np.empty((128, 64), np.float32)
        for hc in range(2):
            h8[:, 32 * hc:32 * (hc + 1)] = 16.0 * h0[l][:, 128 * hc:128 * (hc + 1)].T
            ci[:, 32 * hc:32 * (hc + 1)] = c0[l][:, 128 * hc:128 * (hc + 1)].T
        im[f"h8i_{l}"] = h8.astype(E3M4)
        im[f"ci_{l}"] = ci

    oh = np.zeros((4, 512), np.float32)
    for r in range(4):
        oh[r, 128 * r:128 * (r + 1)] = 1.0
    im["onehot4"] = oh.astype(BF16)

    # input.T [256, T*B] bf16, padded with one body of zeros
    ipT = np.zeros((256, T * B + 2 * HALF * B), dtype=BF16)
    ipT[:, :T * B] = inp.reshape(T * B, 256).T.astype(BF16)
    im["inputT"] = ipT
    return im


def finish_output(fwdT_bf16, T):
    """fwdT [256, T*B] bf16 (= h1 transposed) -> fwd [T, B, H] fp32."""
    fwd = fwdT_bf16.astype(np.float32).T.reshape(T, B, H)
    return np.ascontiguousarray(fwd)


LAST_EXEC_NS = None


def run_device(inputs, T, trace=False, repeats=0):
    import time
    from concourse import bass_utils

    global LAST_EXEC_NS
    nc = build(T)
    im = prep_inputs(inputs, T)
    t0 = time.time()
    res = bass_utils.run_bass_kernel_spmd(nc, [im], [0])
    LAST_EXEC_NS = int((time.time() - t0) * 1e9)
    times = []
    if trace or repeats:
        for _ in range(max(repeats, 3)):
            t0 = time.time()
            res = bass_utils.run_bass_kernel_spmd(nc, [im], [0])
            times.append(time.time() - t0)
        res.exec_time_ns = int(min(times) * 1e9)
        LAST_EXEC_NS = res.exec_time_ns
    fwd = finish_output(res.results[0]["fwdT"], T)
    return fwd, res


def kernel(**inputs):
    T = inputs["input"].shape[0]
    fwd, _ = run_device(inputs, T)
    out = np.empty((T, B, 2 * H), dtype=np.float32)
    out[:, :, :H] = fwd
    out[:, :, H:] = fwd[-1][None]
    return out


def np_ref(inputs, T):
    x_all = np.asarray(inputs["input"], np.float32)
    h = np.asarray(inputs["h0"], np.float32).copy()
    c = np.asarray(inputs["c0"], np.float32).copy()
    Wih = inputs["Wih"]; Whh = inputs["Whh"]
    bih = inputs["bih"]; bhh = inputs["bhh"]
    outs = []
    for t in range(T):
        x = x_all[t] @ inputs["W_init"].T + inputs["b_init"]
        for l in range(2):
            gates = x @ Wih[l].T + bih[l] + h[l] @ Whh[l].T + bhh[l]
            i_, f_, g_, o_ = np.split(gates, 4, axis=-1)
            i_ = 1 / (1 + np.exp(-i_)); f_ = 1 / (1 + np.exp(-f_))
            o_ = 1 / (1 + np.exp(-o_)); g_ = np.tanh(g_)
            c[l] = f_ * c[l] + i_ * g_
            h[l] = o_ * np.tanh(c[l])
            x = h[l]
        outs.append(h[1].copy())
    return np.stack(outs)


if __name__ == "__main__":
    from concourse.bass_interp import CoreSim

    T = int(os.environ.get("SIM_T", "64"))
    rng = np.random.default_rng(0)
    k = 1.0 / np.sqrt(H)
    inputs = {
        "input": rng.standard_normal((T, B, D), dtype=np.float32),
        "W_init": rng.uniform(-k, k, (H, D)).astype(np.float32),
        "b_init": rng.uniform(-k, k, (H,)).astype(np.float32),
        "Wih": rng.uniform(-k, k, (2, 4 * H, H)).astype(np.float32),
        "Whh": rng.uniform(-k, k, (2, 4 * H, H)).astype(np.float32),
        "bih": rng.uniform(-k, k, (2, 4 * H)).astype(np.float32),
        "bhh": rng.uniform(-k, k, (2, 4 * H)).astype(np.float32),
        "h0": rng.uniform(-k, k, (2, B, H)).astype(np.float32),
        "c0": rng.uniform(-k, k, (2, B, H)).astype(np.float32),
    }
    expected = np_ref(inputs, T)

    nc = build(T)
    sim = CoreSim(nc, trace=os.environ.get("SIM_TRACE", "0") == "1")
    im = prep_inputs(inputs, T)
    for name, arr in im.items():
        sim.tensor(name)[:] = arr
    sim.simulate()
    got = finish_output(np.asarray(sim.tensor("fwdT")), T)
    err = np.abs(got - expected).max() / (np.abs(expected).max() + 1e-9)
    print("SIM time ns:", sim.time, " ns/step:", sim.time / T)
    print("SIM max-rel err:", err)
    print("sample got", got[3, 0, :4], "exp", expected[3, 0, :4])
